# revision 12
# baseline (speedup 1.0000x reference)
"""DecisionVQVAE forward on 8 Trainium2 NeuronCores (Bass/Tile).

Data-parallel over the batch dim: 32 batches -> 4 per core (8192 tokens/core).
Weights + codebook replicated; host pre-transposes x to feature-major and
pre-formats weights. Per-core pipeline (16 macro-tiles of 512 tokens):

  encoder (f32r matmuls, feature-major)  -> z
  VQ scores s = 2 z.c - ||c||^2 (token-major, z as lhsT)
  argmax via DVE max/max_index (exact top-1 index)
  one-hot = (s == max) -> PE transpose (bf16) -> q = CB^T @ onehot (bf16)
  decoder (bf16) with L4 flipped back to token-major (h2 as lhsT)
  commit loss via sum(z^2) - sum(max)  [exact algebra for sum (z-q)^2]
"""
import sys

if "/opt/trn_rl_repo" not in sys.path:
    sys.path.insert(0, "/opt/trn_rl_repo")

import ml_dtypes
import numpy as np

import concourse.bass as bass  # noqa: F401
import concourse.tile as tile
from concourse import bacc, mybir
from concourse.bass_utils import run_bass_kernel_spmd

F32 = mybir.dt.float32
F32R = mybir.dt.float32r
BF16 = mybir.dt.bfloat16
AF = mybir.ActivationFunctionType
OP = mybir.AluOpType

N_CORES = 8
B, N, D, H, C, K = 32, 2048, 768, 512, 256, 512
TOK = B * N // N_CORES        # 8192 tokens per core
T = 512                       # tokens per macro tile
N_MACRO = TOK // T            # 16
N_CHUNK = TOK // 128          # 64 chunks of 128 tokens
DK, HK, CK, KK = D // 128, H // 128, C // 128, K // 128  # 6,4,2,4


def _build_nc(add_b4=True):
    nc = bacc.Bacc("TRN2", target_bir_lowering=False, debug=False,
                   num_devices=N_CORES)

    # --- DRAM I/O ---
    xt_d = nc.dram_tensor("xt", [128, DK, TOK], F32R, kind="ExternalInput").ap()
    w1_d = nc.dram_tensor("w1", [128, DK, H], F32R, kind="ExternalInput").ap()
    w2_d = nc.dram_tensor("w2", [128, HK, C], F32R, kind="ExternalInput").ap()
    w3_d = nc.dram_tensor("w3", [128, CK, H], BF16, kind="ExternalInput").ap()
    w4_d = nc.dram_tensor("w4", [128, HK, D], BF16, kind="ExternalInput").ap()
    cb_d = nc.dram_tensor("cb", [128, KK, C], BF16, kind="ExternalInput").ap()
    cb2t_d = nc.dram_tensor("cb2t", [128, CK, K], F32R, kind="ExternalInput").ap()
    csq_d = nc.dram_tensor("csq", [128, K], F32, kind="ExternalInput").ap()
    b1_d = nc.dram_tensor("b1", [128, HK], F32, kind="ExternalInput").ap()
    b2_d = nc.dram_tensor("b2", [128, CK], F32, kind="ExternalInput").ap()
    b3_d = nc.dram_tensor("b3", [128, HK], F32, kind="ExternalInput").ap()
    b4_d = nc.dram_tensor("b4rep", [128, D], F32, kind="ExternalInput").ap()
    ident_d = nc.dram_tensor("ident", [128, 128], BF16, kind="ExternalInput").ap()

    recon_d = nc.dram_tensor("recon", [TOK, D], F32, kind="ExternalOutput").ap()
    idx_d = nc.dram_tensor("indices", [TOK], mybir.dt.int32,
                           kind="ExternalOutput").ap()
    commit_d = nc.dram_tensor("commit", [1, 1], F32, kind="ExternalOutput").ap()


    recon3 = recon_d.rearrange("(o p) m -> p o m", p=128)
    idx2 = idx_d.rearrange("(c t) -> t c", t=128)        # [128, 64]

    with tile.TileContext(nc) as tc:
        with (
            tc.tile_pool(name="wgt", bufs=1) as wgt,
            tc.tile_pool(name="pers", bufs=1) as pers,
            tc.tile_pool(name="io", bufs=2) as io,
            tc.tile_pool(name="mid", bufs=2) as mid,
            tc.tile_pool(name="mid1", bufs=2) as mid1,
            tc.tile_pool(name="one", bufs=1) as one,
            tc.tile_pool(name="ps", bufs=4, space="PSUM") as ps,
            tc.tile_pool(name="ps768", bufs=2, space="PSUM") as ps768,
            tc.tile_pool(name="dram", bufs=1, space="DRAM") as dram,
        ):
            # --- resident weights ---
            w1 = wgt.tile([128, DK, H], F32R, tag="w1")
            w2 = wgt.tile([128, HK, C], F32R, tag="w2")
            w3 = wgt.tile([128, CK, H], BF16, tag="w3")
            w4 = wgt.tile([128, HK, D], BF16, tag="w4")
            cb = wgt.tile([128, KK, C], BF16, tag="cb")
            cb2t = wgt.tile([128, CK, K], F32R, tag="cb2t")
            csq = wgt.tile([128, K], F32, tag="csq")
            b1 = wgt.tile([128, HK], F32, tag="b1")
            b2 = wgt.tile([128, CK], F32, tag="b2")
            b3 = wgt.tile([128, HK], F32, tag="b3")
            b4 = wgt.tile([128, D], F32, tag="b4")
            ident = wgt.tile([128, 128], BF16, tag="ident")
            for sb_t, dr in ((w1, w1_d), (w2, w2_d), (w3, w3_d), (w4, w4_d),
                             (cb, cb_d), (cb2t, cb2t_d), (csq, csq_d),
                             (b1, b1_d), (b2, b2_d), (b3, b3_d), (b4, b4_d),
                             (ident, ident_d)):
                nc.sync.dma_start(sb_t[:], dr[:])

            # --- persistent staging ---
            idx_stage = pers.tile([128, N_CHUNK, 8], mybir.dt.uint32, tag="idxs")
            m_stage = pers.tile([128, N_CHUNK, 8], F32, tag="mstage")
            acc = pers.tile([128, N_MACRO * CK], F32, tag="acc")

            for mac in range(N_MACRO):
                # ---- load x^T macro tile [128, 6, 512] (host pre-transposed) ----
                xt = io.tile([128, DK, T], F32R, tag="xt")
                nc.sync.dma_start(xt[:], xt_d[:, :, mac * T:(mac + 1) * T])

                # ---- L1: h1^T = relu(W1^T x^T + b1)  [128, 4, 512] ----
                h1 = mid1.tile([128, HK, T], F32R, tag="h1")
                for hk in range(HK):
                    ph = ps.tile([128, T], F32, tag="ps")
                    for dk in range(DK):
                        nc.tensor.matmul(
                            ph[:], w1[:, dk, hk * 128:(hk + 1) * 128],
                            xt[:, dk, :], start=(dk == 0), stop=(dk == DK - 1))
                    nc.scalar.activation(h1[:, hk, :], ph[:], AF.Relu,
                                         bias=b1[:, hk:hk + 1])

                # ---- L2: z^T = W2^T h1^T + b2  [128, 2, 512] ----
                z = mid.tile([128, CK, T], F32R, tag="z")
                for ck in range(CK):
                    pz = ps.tile([128, T], F32, tag="ps")
                    for hk in range(HK):
                        nc.tensor.matmul(
                            pz[:], w2[:, hk, ck * 128:(ck + 1) * 128],
                            h1[:, hk, :], start=(hk == 0), stop=(hk == HK - 1))
                    nc.scalar.activation(z[:, ck, :], pz[:], AF.Identity,
                                         bias=b2[:, ck:ck + 1])

                # ---- VQ scores s[t,k] = 2 z.c_k - ||c_k||^2 (T-major) ----
                s = mid.tile([128, 4, K], F32, tag="s")
                oh = mid1.tile([128, 4, K], BF16, tag="oh")
                for tk in range(4):
                    ch = mac * 4 + tk
                    pssc = ps.tile([128, K], F32, tag="ps")
                    for ck in range(CK):
                        nc.tensor.matmul(
                            pssc[:], z[:, ck, tk * 128:(tk + 1) * 128],
                            cb2t[:, ck, :], start=(ck == 0), stop=(ck == CK - 1))
                    nc.vector.tensor_tensor(s[:, tk, :], pssc[:], csq[:],
                                            OP.subtract)
                    nc.vector.max(m_stage[:, ch, :], s[:, tk, :])
                    nc.vector.max_index(idx_stage[:, ch, :],
                                        m_stage[:, ch, :], s[:, tk, :])
                    nc.vector.tensor_scalar(oh[:, tk, :], s[:, tk, :],
                                            m_stage[:, ch, 0:1], None,
                                            OP.is_equal)

                # ---- transpose onehot -> ohT [128, 4, 512] bf16 (K-major) ----
                oht = mid1.tile([128, KK, T], BF16, tag="oht")
                for kk in range(KK):
                    pot = ps.tile([128, T], F32, tag="ps")
                    potb = pot[:].bitcast(BF16)[:, :T]
                    for tk in range(4):
                        nc.tensor.transpose(
                            potb[:, tk * 128:(tk + 1) * 128],
                            oh[:, tk, kk * 128:(kk + 1) * 128],
                            ident[:],
                        )
                    nc.scalar.copy(oht[:, kk, :], potb[:])

                # ---- q^T = CB^T onehot^T  [128, 2, 512] bf16 ----
                qt = mid.tile([128, CK, T], BF16, tag="qt")
                for ck in range(CK):
                    pq = ps.tile([128, T], F32, tag="ps")
                    for kk in range(KK):
                        nc.tensor.matmul(
                            pq[:], cb[:, kk, ck * 128:(ck + 1) * 128],
                            oht[:, kk, :], start=(kk == 0), stop=(kk == KK - 1))
                    nc.scalar.copy(qt[:, ck, :], pq[:])

                # ---- commit partial: sum z^2 (sum(z-q)^2 = sum z^2 - sum m) ----
                for ck in range(CK):
                    scr = one.tile([128, T], F32, tag="scr")
                    nc.scalar.activation(scr[:], z[:, ck, :], AF.Square,
                                         accum_out=acc[:, mac * CK + ck:
                                                       mac * CK + ck + 1])

                # ---- L3: h2^T = relu(W3^T q^T + b3)  bf16 ----
                h2 = mid1.tile([128, HK, T], BF16, tag="h2")
                for hk in range(HK):
                    ph2 = ps.tile([128, T], F32, tag="ps")
                    for ck in range(CK):
                        nc.tensor.matmul(
                            ph2[:], w3[:, ck, hk * 128:(hk + 1) * 128],
                            qt[:, ck, :], start=(ck == 0), stop=(ck == CK - 1))
                    nc.scalar.activation(h2[:, hk, :], ph2[:], AF.Relu,
                                         bias=b3[:, hk:hk + 1])

                # ---- L4 (flip to T-major): recon[t, d] = h2 W4 + b4 ----
                recon_sb = io.tile([128, 4, D], F32, tag="recon")
                for tk in range(4):
                    pr = ps768.tile([128, D], F32, tag="ps768")
                    for lo, hi in ((0, 512), (512, 768)):
                        for hk in range(HK):
                            nc.tensor.matmul(
                                pr[:, lo:hi],
                                h2[:, hk, tk * 128:(tk + 1) * 128],
                                w4[:, hk, lo:hi],
                                start=(hk == 0), stop=(hk == HK - 1))
                    if add_b4:
                        nc.vector.tensor_tensor(recon_sb[:, tk, :], pr[:],
                                                b4[:], OP.add)
                    elif tk % 2 == 0:
                        nc.scalar.copy(recon_sb[:, tk, :], pr[:])
                    else:
                        nc.vector.tensor_copy(recon_sb[:, tk, :], pr[:])
                nc.sync.dma_start(recon3[:, mac * 4:(mac + 1) * 4, :],
                                  recon_sb[:])

            # ---- epilogue: indices + commit ----
            nc.sync.dma_start(idx2[:], idx_stage[:, :, 0].bitcast(mybir.dt.int32))
            acc_red = pers.tile([128, 1], F32, tag="accred")
            nc.vector.tensor_reduce(acc_red[:], acc[:], mybir.AxisListType.X,
                                    OP.add)
            m_sum = pers.tile([128, 1], F32, tag="msum")
            nc.vector.tensor_reduce(m_sum[:], m_stage[:, :, 0],
                                    mybir.AxisListType.X, OP.add)
            nc.vector.tensor_tensor(acc_red[:], acc_red[:], m_sum[:],
                                    OP.subtract)
            acc_bounce = dram.tile([128, 1], F32, tag="accb")
            nc.sync.dma_start(acc_bounce[:], acc_red[:])
            acc_row = pers.tile([1, 128], F32, tag="accrow")
            nc.sync.dma_start(acc_row[:], acc_bounce[:].rearrange("p o -> o p"))
            commit_sb = pers.tile([1, 1], F32, tag="commit")
            nc.vector.tensor_reduce(commit_sb[:], acc_row[:],
                                    mybir.AxisListType.X, OP.add)
            nc.sync.dma_start(commit_d[:], commit_sb[:])


    nc.finalize()
    return nc


_NC = {}


def _get_nc(add_b4=True):
    if add_b4 not in _NC:
        _NC[add_b4] = _build_nc(add_b4)
    return _NC[add_b4]


def _prep_weights(W1, b1v, W2, b2v, W3, b3v, W4, b4v, codebook):
    f = np.float32
    bf = ml_dtypes.bfloat16
    cbf = np.ascontiguousarray(codebook, dtype=f)
    csq = (cbf.astype(np.float64) ** 2).sum(axis=1).astype(f)  # [K]
    wm = {
        "w1": np.ascontiguousarray(W1.reshape(DK, 128, H).transpose(1, 0, 2), f),
        "w2": np.ascontiguousarray(W2.reshape(HK, 128, C).transpose(1, 0, 2), f),
        "w3": np.ascontiguousarray(W3.reshape(CK, 128, H).transpose(1, 0, 2), bf),
        "w4": np.ascontiguousarray(W4.reshape(HK, 128, D).transpose(1, 0, 2), bf),
        "cb": np.ascontiguousarray(cbf.reshape(KK, 128, C).transpose(1, 0, 2), bf),
        "cb2t": np.ascontiguousarray(
            (2.0 * cbf.T).reshape(CK, 128, K).transpose(1, 0, 2), f),
        "csq": np.ascontiguousarray(np.broadcast_to(csq[None, :], (128, K)), f),
        "b1": np.ascontiguousarray(np.asarray(b1v, f).reshape(HK, 128).T, f),
        "b2": np.ascontiguousarray(np.asarray(b2v, f).reshape(CK, 128).T, f),
        "b3": np.ascontiguousarray(np.asarray(b3v, f).reshape(HK, 128).T, f),
        "b4rep": np.ascontiguousarray(
            np.broadcast_to(np.asarray(b4v, f)[None, :], (128, D)), f),
        "ident": np.eye(128, dtype=bf),
    }
    return wm


def _run(inputs, trace=False, **kw):
    b4v = np.asarray(inputs["b4"], np.float32)
    nc = _get_nc(add_b4=bool(np.any(b4v)))
    x = np.asarray(inputs["x"], np.float32)  # [32,2048,768]
    wm = _prep_weights(
        np.asarray(inputs["W1"], np.float32), np.asarray(inputs["b1"], np.float32),
        np.asarray(inputs["W2"], np.float32), np.asarray(inputs["b2"], np.float32),
        np.asarray(inputs["W3"], np.float32), np.asarray(inputs["b3"], np.float32),
        np.asarray(inputs["W4"], np.float32), np.asarray(inputs["b4"], np.float32),
        np.asarray(inputs["codebook"], np.float32))
    bpc = B // N_CORES
    in_maps = []
    for c in range(N_CORES):
        m = dict(wm)
        xs = x[c * bpc:(c + 1) * bpc].reshape(TOK, D)
        # host-side transpose to feature-major [768, 8192] -> [128, 6, 8192]
        m["xt"] = np.ascontiguousarray(
            xs.T.reshape(DK, 128, TOK).transpose(1, 0, 2))
        in_maps.append(m)
    res = run_bass_kernel_spmd(nc, in_maps, core_ids=list(range(N_CORES)),
                               trace=trace, **kw)
    recon = np.stack([res.results[c]["recon"] for c in range(N_CORES)])
    recon = recon.reshape(B, N, D)
    indices = np.stack([res.results[c]["indices"] for c in range(N_CORES)])
    indices = indices.reshape(B, N).astype(np.int32)
    commit = np.float32(
        sum(float(res.results[c]["commit"][0, 0]) for c in range(N_CORES))
        / (B * N * C))
    return (recon, indices, commit), res


def kernel(**inputs):
    out, _ = _run(inputs, trace=False)
    return out


# revision 13
# speedup vs baseline: 1.3352x; 1.3352x over previous
"""DecisionVQVAE forward on 8 Trainium2 NeuronCores (Bass/Tile).

Data-parallel over the batch dim: 32 batches -> 4 per core (8192 tokens/core).
Weights + codebook replicated; host pre-transposes x to feature-major and
pre-formats weights. Per-core pipeline (16 macro-tiles of 512 tokens):

  encoder (f32r matmuls, feature-major)  -> z
  VQ scores s = 2 z.c - ||c||^2 (token-major, z as lhsT)
  argmax via DVE max/max_index (exact top-1 index)
  one-hot = (s == max) -> PE transpose (bf16) -> q = CB^T @ onehot (bf16)
  decoder (bf16) with L4 flipped back to token-major (h2 as lhsT)
  commit loss via sum(z^2) - sum(max)  [exact algebra for sum (z-q)^2]
"""
import sys

if "/opt/trn_rl_repo" not in sys.path:
    sys.path.insert(0, "/opt/trn_rl_repo")

import ml_dtypes
import numpy as np

import concourse.bass as bass  # noqa: F401
import concourse.tile as tile
from concourse import bacc, mybir
from concourse.bass_utils import run_bass_kernel_spmd

F32 = mybir.dt.float32
F32R = mybir.dt.float32r
BF16 = mybir.dt.bfloat16
AF = mybir.ActivationFunctionType
OP = mybir.AluOpType

N_CORES = 8
B, N, D, H, C, K = 32, 2048, 768, 512, 256, 512
TOK = B * N // N_CORES        # 8192 tokens per core
T = 512                       # tokens per macro tile
N_MACRO = TOK // T            # 16
N_CHUNK = TOK // 128          # 64 chunks of 128 tokens
DK, HK, CK, KK = D // 128, H // 128, C // 128, K // 128  # 6,4,2,4


def _build_nc(add_b4=True):
    nc = bacc.Bacc("TRN2", target_bir_lowering=False, debug=False,
                   num_devices=N_CORES)

    # --- DRAM I/O ---
    xt_d = nc.dram_tensor("xt", [128, DK, TOK], F32R, kind="ExternalInput").ap()
    w1_d = nc.dram_tensor("w1", [128, DK, H], F32R, kind="ExternalInput").ap()
    w2_d = nc.dram_tensor("w2", [128, HK, C], F32R, kind="ExternalInput").ap()
    w3_d = nc.dram_tensor("w3", [128, CK, H], BF16, kind="ExternalInput").ap()
    w4_d = nc.dram_tensor("w4", [128, HK, D], BF16, kind="ExternalInput").ap()
    cb_d = nc.dram_tensor("cb", [128, KK, C], BF16, kind="ExternalInput").ap()
    cb2t_d = nc.dram_tensor("cb2t", [128, CK, K], F32R, kind="ExternalInput").ap()
    csq_d = nc.dram_tensor("csq", [128, K], F32, kind="ExternalInput").ap()
    b1_d = nc.dram_tensor("b1", [128, HK], F32, kind="ExternalInput").ap()
    b2_d = nc.dram_tensor("b2", [128, CK], F32, kind="ExternalInput").ap()
    b3_d = nc.dram_tensor("b3", [128, HK], F32, kind="ExternalInput").ap()
    b4_d = nc.dram_tensor("b4rep", [128, D], F32, kind="ExternalInput").ap()
    ident_d = nc.dram_tensor("ident", [128, 128], BF16, kind="ExternalInput").ap()

    recon_d = nc.dram_tensor("recon", [TOK, D], F32, kind="ExternalOutput").ap()
    idx_d = nc.dram_tensor("indices", [TOK], mybir.dt.int32,
                           kind="ExternalOutput").ap()
    commit_d = nc.dram_tensor("commit", [1, 1], F32, kind="ExternalOutput").ap()


    recon3 = recon_d.rearrange("(o p) m -> p o m", p=128)
    idx2 = idx_d.rearrange("(c t) -> t c", t=128)        # [128, 64]

    with tile.TileContext(nc) as tc:
        with (
            tc.tile_pool(name="wgt", bufs=1) as wgt,
            tc.tile_pool(name="pers", bufs=1) as pers,
            tc.tile_pool(name="io", bufs=2) as io,
            tc.tile_pool(name="mid", bufs=2) as mid,
            tc.tile_pool(name="mid1", bufs=2) as mid1,
            tc.tile_pool(name="one", bufs=1) as one,
            tc.tile_pool(name="ps", bufs=4, space="PSUM") as ps,
            tc.tile_pool(name="ps768", bufs=2, space="PSUM") as ps768,
            tc.tile_pool(name="dram", bufs=1, space="DRAM") as dram,
        ):
            # --- resident weights ---
            w1 = wgt.tile([128, DK, H], F32R, tag="w1")
            w2 = wgt.tile([128, HK, C], F32R, tag="w2")
            w3 = wgt.tile([128, CK, H], BF16, tag="w3")
            w4 = wgt.tile([128, HK, D], BF16, tag="w4")
            cb = wgt.tile([128, KK, C], BF16, tag="cb")
            cb2t = wgt.tile([128, CK, K], F32R, tag="cb2t")
            csq = wgt.tile([128, K], F32, tag="csq")
            b1 = wgt.tile([128, HK], F32, tag="b1")
            b2 = wgt.tile([128, CK], F32, tag="b2")
            b3 = wgt.tile([128, HK], F32, tag="b3")
            b4 = wgt.tile([128, D], F32, tag="b4")
            ident = wgt.tile([128, 128], BF16, tag="ident")
            for sb_t, dr in ((w1, w1_d), (w2, w2_d), (w3, w3_d), (w4, w4_d),
                             (cb, cb_d), (cb2t, cb2t_d), (csq, csq_d),
                             (b1, b1_d), (b2, b2_d), (b3, b3_d), (b4, b4_d),
                             (ident, ident_d)):
                nc.sync.dma_start(sb_t[:], dr[:])

            # --- persistent staging ---
            idx_stage = pers.tile([128, N_CHUNK, 8], mybir.dt.uint32, tag="idxs")
            m_stage = pers.tile([128, N_CHUNK, 8], F32, tag="mstage")
            acc = pers.tile([128, N_MACRO * CK], F32, tag="acc")

            def encode(mac):
                # ---- load x^T macro tile + L1 + L2 -> z ----
                xt = io.tile([128, DK, T], F32R, tag="xt", name=f"xt{mac}")
                nc.sync.dma_start(xt[:], xt_d[:, :, mac * T:(mac + 1) * T])
                h1 = mid1.tile([128, HK, T], F32R, tag="h1", name=f"h1_{mac}")
                for hk in range(HK):
                    ph = ps.tile([128, T], F32, tag="ps", name=f"ph{mac}_{hk}")
                    for dk in range(DK):
                        nc.tensor.matmul(
                            ph[:], w1[:, dk, hk * 128:(hk + 1) * 128],
                            xt[:, dk, :], start=(dk == 0), stop=(dk == DK - 1))
                    nc.scalar.activation(h1[:, hk, :], ph[:], AF.Relu,
                                         bias=b1[:, hk:hk + 1])
                z = mid.tile([128, CK, T], F32R, tag="z", name=f"z{mac}")
                for ck in range(CK):
                    pz = ps.tile([128, T], F32, tag="ps", name=f"pz{mac}_{ck}")
                    for hk in range(HK):
                        nc.tensor.matmul(
                            pz[:], w2[:, hk, ck * 128:(ck + 1) * 128],
                            h1[:, hk, :], start=(hk == 0), stop=(hk == HK - 1))
                    nc.scalar.activation(z[:, ck, :], pz[:], AF.Identity,
                                         bias=b2[:, ck:ck + 1])
                return z

            def vq_front(mac, z):
                # ---- scores s = 2 z.c - ||c||^2, argmax, onehot ----
                s = mid.tile([128, 4, K], F32, tag="s", name=f"s{mac}")
                oh = mid1.tile([128, 4, K], BF16, tag="oh", name=f"oh{mac}")
                for tk in range(4):
                    ch = mac * 4 + tk
                    pssc = ps.tile([128, K], F32, tag="ps", name=f"pss{mac}_{tk}")
                    for ck in range(CK):
                        nc.tensor.matmul(
                            pssc[:], z[:, ck, tk * 128:(tk + 1) * 128],
                            cb2t[:, ck, :], start=(ck == 0), stop=(ck == CK - 1))
                    nc.vector.tensor_tensor(s[:, tk, :], pssc[:], csq[:],
                                            OP.subtract)
                    nc.vector.max(m_stage[:, ch, :], s[:, tk, :])
                    nc.vector.max_index(idx_stage[:, ch, :],
                                        m_stage[:, ch, :], s[:, tk, :])
                    nc.vector.tensor_scalar(oh[:, tk, :], s[:, tk, :],
                                            m_stage[:, ch, 0:1], None,
                                            OP.is_equal)
                return oh

            def back_half(mac, z, oh):
                # ---- transpose onehot, q, commit, L3, L4, store ----
                oht = mid1.tile([128, KK, T], BF16, tag="oht", name=f"oht{mac}")
                for kk in range(KK):
                    pot = ps.tile([128, T], F32, tag="ps", name=f"pot{mac}_{kk}")
                    potb = pot[:].bitcast(BF16)[:, :T]
                    for tk in range(4):
                        nc.tensor.transpose(
                            potb[:, tk * 128:(tk + 1) * 128],
                            oh[:, tk, kk * 128:(kk + 1) * 128],
                            ident[:],
                        )
                    nc.scalar.copy(oht[:, kk, :], potb[:])
                qt = mid.tile([128, CK, T], BF16, tag="qt", name=f"qt{mac}")
                for ck in range(CK):
                    pq = ps.tile([128, T], F32, tag="ps", name=f"pq{mac}_{ck}")
                    for kk in range(KK):
                        nc.tensor.matmul(
                            pq[:], cb[:, kk, ck * 128:(ck + 1) * 128],
                            oht[:, kk, :], start=(kk == 0), stop=(kk == KK - 1))
                    nc.scalar.copy(qt[:, ck, :], pq[:])
                for ck in range(CK):
                    scr = one.tile([128, T], F32, tag="scr", name=f"scr{mac}_{ck}")
                    nc.scalar.activation(scr[:], z[:, ck, :], AF.Square,
                                         accum_out=acc[:, mac * CK + ck:
                                                       mac * CK + ck + 1])
                h2 = mid1.tile([128, HK, T], BF16, tag="h2", name=f"h2_{mac}")
                for hk in range(HK):
                    ph2 = ps.tile([128, T], F32, tag="ps", name=f"ph2{mac}_{hk}")
                    for ck in range(CK):
                        nc.tensor.matmul(
                            ph2[:], w3[:, ck, hk * 128:(hk + 1) * 128],
                            qt[:, ck, :], start=(ck == 0), stop=(ck == CK - 1))
                    nc.scalar.activation(h2[:, hk, :], ph2[:], AF.Relu,
                                         bias=b3[:, hk:hk + 1])
                recon_sb = io.tile([128, 4, D], F32, tag="recon",
                                   name=f"rc{mac}")
                for tk in range(4):
                    pr = ps768.tile([128, D], F32, tag="ps768",
                                    name=f"pr{mac}_{tk}")
                    for lo, hi in ((0, 512), (512, 768)):
                        for hk in range(HK):
                            nc.tensor.matmul(
                                pr[:, lo:hi],
                                h2[:, hk, tk * 128:(tk + 1) * 128],
                                w4[:, hk, lo:hi],
                                start=(hk == 0), stop=(hk == HK - 1))
                    if add_b4:
                        nc.vector.tensor_tensor(recon_sb[:, tk, :], pr[:],
                                                b4[:], OP.add)
                    elif tk % 2 == 0:
                        nc.scalar.copy(recon_sb[:, tk, :], pr[:])
                    else:
                        nc.vector.tensor_copy(recon_sb[:, tk, :], pr[:])
                nc.sync.dma_start(recon3[:, mac * 4:(mac + 1) * 4, :],
                                  recon_sb[:])

            # software pipeline: encode(m+1) fills the PE gap while the DVE
            # argmax chain of macro m runs
            prev = None
            z = encode(0)
            for mac in range(N_MACRO):
                oh = vq_front(mac, z)
                if mac + 1 < N_MACRO:
                    z_next = encode(mac + 1)
                else:
                    z_next = None
                back_half(mac, z, oh)
                z = z_next

            # ---- epilogue: indices + commit ----
            nc.sync.dma_start(idx2[:], idx_stage[:, :, 0].bitcast(mybir.dt.int32))
            acc_red = pers.tile([128, 1], F32, tag="accred")
            nc.vector.tensor_reduce(acc_red[:], acc[:], mybir.AxisListType.X,
                                    OP.add)
            m_sum = pers.tile([128, 1], F32, tag="msum")
            nc.vector.tensor_reduce(m_sum[:], m_stage[:, :, 0],
                                    mybir.AxisListType.X, OP.add)
            nc.vector.tensor_tensor(acc_red[:], acc_red[:], m_sum[:],
                                    OP.subtract)
            acc_bounce = dram.tile([128, 1], F32, tag="accb")
            nc.sync.dma_start(acc_bounce[:], acc_red[:])
            acc_row = pers.tile([1, 128], F32, tag="accrow")
            nc.sync.dma_start(acc_row[:], acc_bounce[:].rearrange("p o -> o p"))
            commit_sb = pers.tile([1, 1], F32, tag="commit")
            nc.vector.tensor_reduce(commit_sb[:], acc_row[:],
                                    mybir.AxisListType.X, OP.add)
            nc.sync.dma_start(commit_d[:], commit_sb[:])


    nc.finalize()
    return nc


_NC = {}


def _get_nc(add_b4=True):
    if add_b4 not in _NC:
        _NC[add_b4] = _build_nc(add_b4)
    return _NC[add_b4]


def _prep_weights(W1, b1v, W2, b2v, W3, b3v, W4, b4v, codebook):
    f = np.float32
    bf = ml_dtypes.bfloat16
    cbf = np.ascontiguousarray(codebook, dtype=f)
    csq = (cbf.astype(np.float64) ** 2).sum(axis=1).astype(f)  # [K]
    wm = {
        "w1": np.ascontiguousarray(W1.reshape(DK, 128, H).transpose(1, 0, 2), f),
        "w2": np.ascontiguousarray(W2.reshape(HK, 128, C).transpose(1, 0, 2), f),
        "w3": np.ascontiguousarray(W3.reshape(CK, 128, H).transpose(1, 0, 2), bf),
        "w4": np.ascontiguousarray(W4.reshape(HK, 128, D).transpose(1, 0, 2), bf),
        "cb": np.ascontiguousarray(cbf.reshape(KK, 128, C).transpose(1, 0, 2), bf),
        "cb2t": np.ascontiguousarray(
            (2.0 * cbf.T).reshape(CK, 128, K).transpose(1, 0, 2), f),
        "csq": np.ascontiguousarray(np.broadcast_to(csq[None, :], (128, K)), f),
        "b1": np.ascontiguousarray(np.asarray(b1v, f).reshape(HK, 128).T, f),
        "b2": np.ascontiguousarray(np.asarray(b2v, f).reshape(CK, 128).T, f),
        "b3": np.ascontiguousarray(np.asarray(b3v, f).reshape(HK, 128).T, f),
        "b4rep": np.ascontiguousarray(
            np.broadcast_to(np.asarray(b4v, f)[None, :], (128, D)), f),
        "ident": np.eye(128, dtype=bf),
    }
    return wm


def _run(inputs, trace=False, **kw):
    b4v = np.asarray(inputs["b4"], np.float32)
    nc = _get_nc(add_b4=bool(np.any(b4v)))
    x = np.asarray(inputs["x"], np.float32)  # [32,2048,768]
    wm = _prep_weights(
        np.asarray(inputs["W1"], np.float32), np.asarray(inputs["b1"], np.float32),
        np.asarray(inputs["W2"], np.float32), np.asarray(inputs["b2"], np.float32),
        np.asarray(inputs["W3"], np.float32), np.asarray(inputs["b3"], np.float32),
        np.asarray(inputs["W4"], np.float32), np.asarray(inputs["b4"], np.float32),
        np.asarray(inputs["codebook"], np.float32))
    bpc = B // N_CORES
    in_maps = []
    for c in range(N_CORES):
        m = dict(wm)
        xs = x[c * bpc:(c + 1) * bpc].reshape(TOK, D)
        # host-side transpose to feature-major [768, 8192] -> [128, 6, 8192]
        m["xt"] = np.ascontiguousarray(
            xs.T.reshape(DK, 128, TOK).transpose(1, 0, 2))
        in_maps.append(m)
    res = run_bass_kernel_spmd(nc, in_maps, core_ids=list(range(N_CORES)),
                               trace=trace, **kw)
    recon = np.stack([res.results[c]["recon"] for c in range(N_CORES)])
    recon = recon.reshape(B, N, D)
    indices = np.stack([res.results[c]["indices"] for c in range(N_CORES)])
    indices = indices.reshape(B, N).astype(np.int32)
    commit = np.float32(
        sum(float(res.results[c]["commit"][0, 0]) for c in range(N_CORES))
        / (B * N * C))
    return (recon, indices, commit), res


def kernel(**inputs):
    out, _ = _run(inputs, trace=False)
    return out


# revision 14
# speedup vs baseline: 1.4310x; 1.0717x over previous
"""DecisionVQVAE forward on 8 Trainium2 NeuronCores (Bass/Tile).

Data-parallel over the batch dim: 32 batches -> 4 per core (8192 tokens/core).
Weights + codebook replicated; host pre-transposes x to feature-major and
pre-formats weights. Per-core pipeline (16 macro-tiles of 512 tokens):

  encoder (f32r matmuls, feature-major)  -> z
  VQ scores s = 2 z.c - ||c||^2 (token-major, z as lhsT)
  argmax via DVE max/max_index (exact top-1 index)
  one-hot = (s == max) -> PE transpose (bf16) -> q = CB^T @ onehot (bf16)
  decoder (bf16) with L4 flipped back to token-major (h2 as lhsT)
  commit loss via sum(z^2) - sum(max)  [exact algebra for sum (z-q)^2]
"""
import sys

if "/opt/trn_rl_repo" not in sys.path:
    sys.path.insert(0, "/opt/trn_rl_repo")

import ml_dtypes
import numpy as np

import concourse.bass as bass  # noqa: F401
import concourse.tile as tile
from concourse import bacc, mybir
from concourse.bass_utils import run_bass_kernel_spmd

F32 = mybir.dt.float32
F32R = mybir.dt.float32r
BF16 = mybir.dt.bfloat16
AF = mybir.ActivationFunctionType
OP = mybir.AluOpType

N_CORES = 8
B, N, D, H, C, K = 32, 2048, 768, 512, 256, 512
TOK = B * N // N_CORES        # 8192 tokens per core
T = 512                       # tokens per macro tile
N_MACRO = TOK // T            # 16
N_CHUNK = TOK // 128          # 64 chunks of 128 tokens
DK, HK, CK, KK = D // 128, H // 128, C // 128, K // 128  # 6,4,2,4


def _build_nc(add_b4=True):
    nc = bacc.Bacc("TRN2", target_bir_lowering=False, debug=False,
                   num_devices=N_CORES)

    # --- DRAM I/O ---
    xt_d = nc.dram_tensor("xt", [128, DK, TOK], F32R, kind="ExternalInput").ap()
    w1_d = nc.dram_tensor("w1", [128, DK, H], F32R, kind="ExternalInput").ap()
    w2_d = nc.dram_tensor("w2", [128, HK, C], F32R, kind="ExternalInput").ap()
    w3_d = nc.dram_tensor("w3", [128, CK, H], BF16, kind="ExternalInput").ap()
    w4_d = nc.dram_tensor("w4", [128, HK, D], BF16, kind="ExternalInput").ap()
    cb_d = nc.dram_tensor("cb", [128, KK, C], BF16, kind="ExternalInput").ap()
    cb2t_d = nc.dram_tensor("cb2t", [128, CK, K], F32R, kind="ExternalInput").ap()
    csq_d = nc.dram_tensor("csq", [128, K], F32, kind="ExternalInput").ap()
    b1_d = nc.dram_tensor("b1", [128, HK], F32, kind="ExternalInput").ap()
    b2_d = nc.dram_tensor("b2", [128, CK], F32, kind="ExternalInput").ap()
    b3_d = nc.dram_tensor("b3", [128, HK], F32, kind="ExternalInput").ap()
    b4_d = nc.dram_tensor("b4rep", [128, D], F32, kind="ExternalInput").ap()
    ident_d = nc.dram_tensor("ident", [128, 128], BF16, kind="ExternalInput").ap()

    recon_d = nc.dram_tensor("recon", [TOK, D], F32, kind="ExternalOutput").ap()
    idx_d = nc.dram_tensor("indices", [TOK], mybir.dt.int32,
                           kind="ExternalOutput").ap()
    commit_d = nc.dram_tensor("commit", [1, 1], F32, kind="ExternalOutput").ap()


    recon3 = recon_d.rearrange("(o p) m -> p o m", p=128)
    idx2 = idx_d.rearrange("(c t) -> t c", t=128)        # [128, 64]

    with tile.TileContext(nc) as tc:
        with (
            tc.tile_pool(name="wgt", bufs=1) as wgt,
            tc.tile_pool(name="pers", bufs=1) as pers,
            tc.tile_pool(name="io", bufs=2) as io,
            tc.tile_pool(name="mid", bufs=2) as mid,
            tc.tile_pool(name="mid1", bufs=2) as mid1,
            tc.tile_pool(name="one", bufs=1) as one,
            tc.tile_pool(name="ps", bufs=4, space="PSUM") as ps,
            tc.tile_pool(name="ps768", bufs=2, space="PSUM") as ps768,
            tc.tile_pool(name="dram", bufs=1, space="DRAM") as dram,
        ):
            # --- resident weights ---
            w1 = wgt.tile([128, DK, H], F32R, tag="w1")
            w2 = wgt.tile([128, HK, C], F32R, tag="w2")
            w3 = wgt.tile([128, CK, H], BF16, tag="w3")
            w4 = wgt.tile([128, HK, D], BF16, tag="w4")
            cb = wgt.tile([128, KK, C], BF16, tag="cb")
            cb2t = wgt.tile([128, CK, K], F32R, tag="cb2t")
            csq = wgt.tile([128, K], F32, tag="csq")
            b1 = wgt.tile([128, HK], F32, tag="b1")
            b2 = wgt.tile([128, CK], F32, tag="b2")
            b3 = wgt.tile([128, HK], F32, tag="b3")
            b4 = wgt.tile([128, D], F32, tag="b4")
            ident = wgt.tile([128, 128], BF16, tag="ident")
            for sb_t, dr in ((w1, w1_d), (w2, w2_d), (w3, w3_d), (w4, w4_d),
                             (cb, cb_d), (cb2t, cb2t_d), (csq, csq_d),
                             (b1, b1_d), (b2, b2_d), (b3, b3_d), (b4, b4_d),
                             (ident, ident_d)):
                nc.sync.dma_start(sb_t[:], dr[:])

            # --- persistent staging ---
            idx_stage = pers.tile([128, N_CHUNK, 8], mybir.dt.uint32, tag="idxs")
            m_stage = pers.tile([128, N_CHUNK, 8], F32, tag="mstage")
            acc = pers.tile([128, N_MACRO * CK], F32, tag="acc")

            def encode(mac):
                # ---- load x^T macro tile + L1 + L2 -> z ----
                xt = io.tile([128, DK, T], F32R, tag="xt", name=f"xt{mac}")
                nc.sync.dma_start(xt[:], xt_d[:, :, mac * T:(mac + 1) * T])
                h1 = mid1.tile([128, HK, T], F32R, tag="h1", name=f"h1_{mac}")
                for hk in range(HK):
                    ph = ps.tile([128, T], F32, tag="ps", name=f"ph{mac}_{hk}")
                    for dk in range(DK):
                        nc.tensor.matmul(
                            ph[:], w1[:, dk, hk * 128:(hk + 1) * 128],
                            xt[:, dk, :], start=(dk == 0), stop=(dk == DK - 1))
                    nc.scalar.activation(h1[:, hk, :], ph[:], AF.Relu,
                                         bias=b1[:, hk:hk + 1])
                z = mid.tile([128, CK, T], F32R, tag="z", name=f"z{mac}")
                for ck in range(CK):
                    pz = ps.tile([128, T], F32, tag="ps", name=f"pz{mac}_{ck}")
                    for hk in range(HK):
                        nc.tensor.matmul(
                            pz[:], w2[:, hk, ck * 128:(ck + 1) * 128],
                            h1[:, hk, :], start=(hk == 0), stop=(hk == HK - 1))
                    nc.scalar.activation(z[:, ck, :], pz[:], AF.Identity,
                                         bias=b2[:, ck:ck + 1])
                return z

            def vq_front(mac, z):
                # ---- scores s = 2 z.c - ||c||^2, argmax, onehot ----
                s = mid.tile([128, 4, K], F32, tag="s", name=f"s{mac}")
                oh = mid1.tile([128, 4, K], BF16, tag="oh", name=f"oh{mac}")
                for tk in range(4):
                    ch = mac * 4 + tk
                    pssc = ps.tile([128, K], F32, tag="ps", name=f"pss{mac}_{tk}")
                    for ck in range(CK):
                        nc.tensor.matmul(
                            pssc[:], z[:, ck, tk * 128:(tk + 1) * 128],
                            cb2t[:, ck, :], start=(ck == 0), stop=(ck == CK - 1))
                    nc.vector.tensor_tensor(s[:, tk, :], pssc[:], csq[:],
                                            OP.subtract)
                    nc.vector.max(m_stage[:, ch, :], s[:, tk, :])
                    nc.vector.max_index(idx_stage[:, ch, :],
                                        m_stage[:, ch, :], s[:, tk, :])
                    nc.vector.tensor_scalar(oh[:, tk, :], s[:, tk, :],
                                            m_stage[:, ch, 0:1], None,
                                            OP.is_equal)
                return oh

            def back_half(mac, z, oh):
                # ---- transpose onehot, q, commit, L3, L4, store ----
                oht = mid1.tile([128, KK, T], BF16, tag="oht", name=f"oht{mac}")
                for kk in range(KK):
                    pot = ps.tile([128, T], F32, tag="ps", name=f"pot{mac}_{kk}")
                    potb = pot[:].bitcast(BF16)[:, :T]
                    for tk in range(4):
                        nc.tensor.transpose(
                            potb[:, tk * 128:(tk + 1) * 128],
                            oh[:, tk, kk * 128:(kk + 1) * 128],
                            ident[:],
                        )
                    nc.vector.tensor_copy(oht[:, kk, :], potb[:])
                qt = mid.tile([128, CK, T], BF16, tag="qt", name=f"qt{mac}")
                for ck in range(CK):
                    pq = ps.tile([128, T], F32, tag="ps", name=f"pq{mac}_{ck}")
                    for kk in range(KK):
                        nc.tensor.matmul(
                            pq[:], cb[:, kk, ck * 128:(ck + 1) * 128],
                            oht[:, kk, :], start=(kk == 0), stop=(kk == KK - 1))
                    nc.vector.tensor_copy(qt[:, ck, :], pq[:])
                for ck in range(CK):
                    scr = one.tile([128, T], F32, tag="scr", name=f"scr{mac}_{ck}")
                    nc.scalar.activation(scr[:], z[:, ck, :], AF.Square,
                                         accum_out=acc[:, mac * CK + ck:
                                                       mac * CK + ck + 1])
                h2 = mid1.tile([128, HK, T], BF16, tag="h2", name=f"h2_{mac}")
                for hk in range(HK):
                    ph2 = ps.tile([128, T], F32, tag="ps", name=f"ph2{mac}_{hk}")
                    for ck in range(CK):
                        nc.tensor.matmul(
                            ph2[:], w3[:, ck, hk * 128:(hk + 1) * 128],
                            qt[:, ck, :], start=(ck == 0), stop=(ck == CK - 1))
                    nc.scalar.activation(h2[:, hk, :], ph2[:], AF.Relu,
                                         bias=b3[:, hk:hk + 1])
                recon_sb = io.tile([128, 4, D], F32, tag="recon",
                                   name=f"rc{mac}")
                for tk in range(4):
                    pr = ps768.tile([128, D], F32, tag="ps768",
                                    name=f"pr{mac}_{tk}")
                    for lo, hi in ((0, 512), (512, 768)):
                        for hk in range(HK):
                            nc.tensor.matmul(
                                pr[:, lo:hi],
                                h2[:, hk, tk * 128:(tk + 1) * 128],
                                w4[:, hk, lo:hi],
                                start=(hk == 0), stop=(hk == HK - 1))
                    if add_b4:
                        nc.vector.tensor_tensor(recon_sb[:, tk, :], pr[:],
                                                b4[:], OP.add)
                    elif tk % 2 == 0:
                        nc.scalar.copy(recon_sb[:, tk, :], pr[:])
                    else:
                        nc.vector.tensor_copy(recon_sb[:, tk, :], pr[:])
                nc.sync.dma_start(recon3[:, mac * 4:(mac + 1) * 4, :],
                                  recon_sb[:])

            # software pipeline: encode(m+1) fills the PE gap while the DVE
            # argmax chain of macro m runs
            prev = None
            z = encode(0)
            for mac in range(N_MACRO):
                oh = vq_front(mac, z)
                if mac + 1 < N_MACRO:
                    z_next = encode(mac + 1)
                else:
                    z_next = None
                back_half(mac, z, oh)
                z = z_next

            # ---- epilogue: indices + commit ----
            nc.sync.dma_start(idx2[:], idx_stage[:, :, 0].bitcast(mybir.dt.int32))
            acc_red = pers.tile([128, 1], F32, tag="accred")
            nc.vector.tensor_reduce(acc_red[:], acc[:], mybir.AxisListType.X,
                                    OP.add)
            m_sum = pers.tile([128, 1], F32, tag="msum")
            nc.vector.tensor_reduce(m_sum[:], m_stage[:, :, 0],
                                    mybir.AxisListType.X, OP.add)
            nc.vector.tensor_tensor(acc_red[:], acc_red[:], m_sum[:],
                                    OP.subtract)
            acc_bounce = dram.tile([128, 1], F32, tag="accb")
            nc.sync.dma_start(acc_bounce[:], acc_red[:])
            acc_row = pers.tile([1, 128], F32, tag="accrow")
            nc.sync.dma_start(acc_row[:], acc_bounce[:].rearrange("p o -> o p"))
            commit_sb = pers.tile([1, 1], F32, tag="commit")
            nc.vector.tensor_reduce(commit_sb[:], acc_row[:],
                                    mybir.AxisListType.X, OP.add)
            nc.sync.dma_start(commit_d[:], commit_sb[:])


    nc.finalize()
    return nc


_NC = {}


def _get_nc(add_b4=True):
    if add_b4 not in _NC:
        _NC[add_b4] = _build_nc(add_b4)
    return _NC[add_b4]


def _prep_weights(W1, b1v, W2, b2v, W3, b3v, W4, b4v, codebook):
    f = np.float32
    bf = ml_dtypes.bfloat16
    cbf = np.ascontiguousarray(codebook, dtype=f)
    csq = (cbf.astype(np.float64) ** 2).sum(axis=1).astype(f)  # [K]
    wm = {
        "w1": np.ascontiguousarray(W1.reshape(DK, 128, H).transpose(1, 0, 2), f),
        "w2": np.ascontiguousarray(W2.reshape(HK, 128, C).transpose(1, 0, 2), f),
        "w3": np.ascontiguousarray(W3.reshape(CK, 128, H).transpose(1, 0, 2), bf),
        "w4": np.ascontiguousarray(W4.reshape(HK, 128, D).transpose(1, 0, 2), bf),
        "cb": np.ascontiguousarray(cbf.reshape(KK, 128, C).transpose(1, 0, 2), bf),
        "cb2t": np.ascontiguousarray(
            (2.0 * cbf.T).reshape(CK, 128, K).transpose(1, 0, 2), f),
        "csq": np.ascontiguousarray(np.broadcast_to(csq[None, :], (128, K)), f),
        "b1": np.ascontiguousarray(np.asarray(b1v, f).reshape(HK, 128).T, f),
        "b2": np.ascontiguousarray(np.asarray(b2v, f).reshape(CK, 128).T, f),
        "b3": np.ascontiguousarray(np.asarray(b3v, f).reshape(HK, 128).T, f),
        "b4rep": np.ascontiguousarray(
            np.broadcast_to(np.asarray(b4v, f)[None, :], (128, D)), f),
        "ident": np.eye(128, dtype=bf),
    }
    return wm


def _run(inputs, trace=False, **kw):
    b4v = np.asarray(inputs["b4"], np.float32)
    nc = _get_nc(add_b4=bool(np.any(b4v)))
    x = np.asarray(inputs["x"], np.float32)  # [32,2048,768]
    wm = _prep_weights(
        np.asarray(inputs["W1"], np.float32), np.asarray(inputs["b1"], np.float32),
        np.asarray(inputs["W2"], np.float32), np.asarray(inputs["b2"], np.float32),
        np.asarray(inputs["W3"], np.float32), np.asarray(inputs["b3"], np.float32),
        np.asarray(inputs["W4"], np.float32), np.asarray(inputs["b4"], np.float32),
        np.asarray(inputs["codebook"], np.float32))
    bpc = B // N_CORES
    in_maps = []
    for c in range(N_CORES):
        m = dict(wm)
        xs = x[c * bpc:(c + 1) * bpc].reshape(TOK, D)
        # host-side transpose to feature-major [768, 8192] -> [128, 6, 8192]
        m["xt"] = np.ascontiguousarray(
            xs.T.reshape(DK, 128, TOK).transpose(1, 0, 2))
        in_maps.append(m)
    res = run_bass_kernel_spmd(nc, in_maps, core_ids=list(range(N_CORES)),
                               trace=trace, **kw)
    recon = np.stack([res.results[c]["recon"] for c in range(N_CORES)])
    recon = recon.reshape(B, N, D)
    indices = np.stack([res.results[c]["indices"] for c in range(N_CORES)])
    indices = indices.reshape(B, N).astype(np.int32)
    commit = np.float32(
        sum(float(res.results[c]["commit"][0, 0]) for c in range(N_CORES))
        / (B * N * C))
    return (recon, indices, commit), res


def kernel(**inputs):
    out, _ = _run(inputs, trace=False)
    return out
